# revision 1
# baseline (speedup 1.0000x reference)
"""Trainium2 Bass kernel for nn_BatchMinigrid: batched FPV render.

Strategy (per core, 4096 envs):
- Host packs each padded+pre-rotated variant pixel into ONE byte:
  v = ch0 | ch1<<2 | ch2<<4 | closed<<6   (closed = walls|closed_door).
  4 rot90 variants of the wall-padded 32x32 image, flat [4*4096*1024] u8.
- Host computes the per-env slab start index (linear in pos/dir); the
  kernel gathers one 199-byte slab per env via 32 indirect DMA calls of
  128 descriptors each (the HW SWDGE path only supports single-column
  offset APs; per-call cost is ~1.1us fixed, so this spine is ~45us).
- closed mask = (v >= 64), one DVE op per 512-env supertile.
- Cell-major layout uses 113 rows (zero band 49:64), two parities:
    alpha (even unit): closed rows 0:49,  t rows 64:113
    beta  (odd  unit): t rows 0:49,  closed rows 64:113
  so one PSUM z tile + one ACT tanh [0:113] serves TWO units per step,
  and one PE transpose returns both units' final masks at once.
- 5-step visibility fixed point as fp32 matmuls (bit-stable vs ref),
  final conv in bf16 (sign-exact), mask transposed back env-major.
- Output = mask * packed_byte as int32 [4096,49]; host unpacks channels
  ((m*v)>>2k & 3 == m*ch_k exactly since mask is 0/1).
"""
import os
import numpy as np
import ml_dtypes
from contextlib import ExitStack

import concourse.bass as bass
import concourse.tile as tile
from concourse import mybir
from concourse.bass_utils import run_bass_kernel_spmd
from concourse.masks import make_identity

P = 128
NENV = 4096          # envs per core
NPAIR = 4            # matmul pairs (1024 envs each)
EB = 512             # envs per matmul column block
SLOT = 256           # slab slot stride (bytes, 32B-aligned), slab run = 199
RUN = 199
VARPIX = NENV * 1024  # pixels per variant per core
KR = 113             # contraction rows (49 + 15 zero + 49)
TB = 64              # upper band base

LAST_RESULTS = {}    # test harness introspection


# ----------------------------------------------------------------- waitsplit
def _split_excess_waits(nc, limit=1):
    n_split = 0
    for fn in nc.m.functions:
        for blk in fn.blocks:
            insts = blk.instructions
            i = 0
            while i < len(insts):
                inst = insts[i]
                si = getattr(inst, "sync_info", None)
                if si is not None and si.on_wait and len(si.on_wait) > limit:
                    waits = list(si.on_wait)
                    si.on_wait.clear()
                    si.on_wait.extend(waits[-limit:])
                    rest = waits[:-limit]
                    pos = i
                    for j in range(0, len(rest), limit):
                        nop = mybir.InstNoOp(
                            name=f"{inst.name}_wsplit{j}",
                            engine=inst.engine,
                            bass_nofuse=True,
                            sync_info=mybir.SyncInfo(
                                on_wait=rest[j:j + limit], on_update=[]),
                        )
                        insts.insert(pos, nop)
                        pos += 1
                        i += 1
                        n_split += 1
                i += 1
    return n_split


# ----------------------------------------------------------------- builder
def build_nc():
    f32 = mybir.dt.float32
    bf16 = mybir.dt.bfloat16
    i32 = mybir.dt.int32
    u8 = mybir.dt.uint8
    nc = bass.Bass()

    var = nc.dram_tensor("var", [4 * VARPIX, 1], u8, kind="ExternalInput")
    idxs = nc.dram_tensor("idxs", [P, 32], i32, kind="ExternalInput")
    lhs_it = nc.dram_tensor("lhs_it", [KR, 98], f32, kind="ExternalInput")
    lhs_fin = nc.dram_tensor("lhs_fin", [KR, 49], bf16, kind="ExternalInput")
    w27 = nc.dram_tensor("w27", [KR, 1], f32, kind="ExternalInput")
    out = nc.dram_tensor("out", [NENV, 49], i32, kind="ExternalOutput")

    AP = bass.AP

    with tile.TileContext(nc) as tc, ExitStack() as ctx:
        const = ctx.enter_context(tc.tile_pool(name="const", bufs=1))
        slabp = ctx.enter_context(tc.tile_pool(name="slabp", bufs=1))
        stp = ctx.enter_context(tc.tile_pool(name="stp", bufs=1))
        workp = ctx.enter_context(tc.tile_pool(name="workp", bufs=3))
        thp = ctx.enter_context(tc.tile_pool(name="thp", bufs=3))
        outp = ctx.enter_context(tc.tile_pool(name="outp", bufs=1))
        psAB = ctx.enter_context(tc.tile_pool(name="psAB", bufs=2, space="PSUM"))
        psZ = ctx.enter_context(tc.tile_pool(name="psZ", bufs=4, space="PSUM"))
        psF = ctx.enter_context(tc.tile_pool(name="psF", bufs=1, space="PSUM"))
        psM = ctx.enter_context(tc.tile_pool(name="psM", bufs=1, space="PSUM"))

        TS = nc.vector.tensor_scalar
        TT = nc.vector.tensor_tensor
        GTT = nc.gpsimd.tensor_tensor
        Alu = mybir.AluOpType
        ACTF = mybir.ActivationFunctionType

        # ---------------- index load + gathers first (critical path)
        idx_t = const.tile([P, 32], i32)
        nc.sync.dma_start(out=idx_t[:], in_=idxs[:])

        slabs = []
        with nc.named_scope("gather"):
            for pi in range(NPAIR):
                slab = slabp.tile([P, 8 * SLOT], u8, tag=f"slab{pi}",
                                  name=f"slab{pi}")
                slabs.append(slab)
                for j in range(8):
                    c = 8 * pi + j
                    nc.gpsimd.indirect_dma_start(
                        out=slab[:, j * SLOT: j * SLOT + RUN],
                        out_offset=None,
                        in_=var[:],
                        in_offset=bass.IndirectOffsetOnAxis(
                            ap=idx_t[:, c:c + 1], axis=0),
                    )

        # ---------------- constants
        ident = const.tile([P, P], bf16)
        make_identity(nc, ident[:])
        lhs_it_t = const.tile([P, 98], f32)
        nc.sync.dma_start(out=lhs_it_t[0:KR, :], in_=lhs_it[:])
        lhs_fin_t = const.tile([P, 49], bf16)
        nc.sync.dma_start(out=lhs_fin_t[0:KR, :], in_=lhs_fin[:])
        w27_t = const.tile([P, 1], f32)
        nc.sync.dma_start(out=w27_t[0:KR, :], in_=w27[:])

        NU = 8               # supertiles (512 envs each)
        sts = [None] * NU
        ops = [None] * NU
        mbs = [None] * NU

        # ---------------- front end per supertile (unit = 512 envs)
        def front(u):
            par = u % 2  # 0 = alpha, 1 = beta
            st = stp.tile([P, EB], f32, tag=f"st{u}", name=f"st{u}")
            op_t = stp.tile([P, EB], bf16, tag=f"op{u}", name=f"op{u}")
            sts[u] = st
            ops[u] = op_t
            slab = slabs[u // 2]
            h = u % 2  # unit's half within the pair-slab
            # NOTE: slab pi holds j in [8pi, 8pi+8); unit u covers
            # j in [4u, 4u+4) -> slab index u//2, half u%2.

            tpAB = psAB.tile([P, 2 * EB], bf16, tag="tpAB", name=f"tpAB{u}")

            with nc.named_scope("closed"):
                clA = workp.tile([P, 64 + 4 * 49], bf16, tag="clA",
                                 name=f"clA{u}")
                sb = slab[:]
                vview = AP(tensor=sb.tensor,
                           offset=sb.offset + h * 4 * SLOT,
                           ap=[sb.ap[0], [SLOT, 4], [32, 7], [1, 7]])
                TS(out=clA[:, 64:260].rearrange("p (g x) -> p g x", g=4),
                   in0=vview, scalar1=64.0, scalar2=None, op0=Alu.is_ge)
            with nc.named_scope("transpose_in"):
                for g in range(4):
                    cb = g * P
                    nc.tensor.transpose(
                        out=tpAB[0:49, cb:cb + P],
                        in_=clA[:, 64 + g * 49: 64 + (g + 1) * 49],
                        identity=ident[:])
                    nc.tensor.transpose(
                        out=tpAB[0:KR, EB + cb:EB + cb + P],
                        in_=clA[:, g * 49: g * 49 + KR],
                        identity=ident[:])

            with nc.named_scope("front_fin"):
                nc.vector.memset(st[32:TB, :], 0.0)
                if par == 0:
                    # closed rows 0:49 from tpA; open rows 64:113 from tpB
                    nc.scalar.copy(out=st[0:49, :], in_=tpAB[0:49, 0:EB])
                    TS(out=op_t[TB:KR, :], in0=tpAB[TB:KR, EB:2 * EB],
                       scalar1=-1.0, scalar2=1.0, op0=Alu.mult, op1=Alu.add)
                    nc.scalar.activation(
                        out=st[TB:KR, :], in_=op_t[TB:KR, :],
                        func=ACTF.Copy, scale=w27_t[TB:KR, :])
                else:
                    # closed rows 64:113 from tpB; open rows 0:49 from tpA
                    nc.scalar.copy(out=st[TB:KR, :], in_=tpAB[TB:KR, EB:2 * EB])
                    TS(out=op_t[0:49, :], in0=tpAB[0:49, 0:EB],
                       scalar1=-1.0, scalar2=1.0, op0=Alu.mult, op1=Alu.add)
                    nc.scalar.activation(
                        out=st[0:49, :], in_=op_t[0:49, :],
                        func=ACTF.Copy, scale=w27_t[0:49, :])

        # ---------------- iterations (l-major across quads)
        def iter_l(l, q):
            ua, ub = 2 * q, 2 * q + 1
            fn = ACTF.Tanh if l < 5 else ACTF.Relu
            with nc.named_scope(f"iter{l}"):
                zq = psZ.tile([P, EB], f32, tag="z", name=f"z{q}_{l}")
                th = thp.tile([P, EB], f32, tag="th", name=f"th{q}_{l}")
                if l < 5:
                    outa, outb = sts[ua], sts[ub]
                else:
                    outa = stp.tile([P, EB], bf16, tag=f"mb{ua}",
                                    name=f"mb{ua}")
                    outb = stp.tile([P, EB], bf16, tag=f"mb{ub}",
                                    name=f"mb{ub}")
                    mbs[ua], mbs[ub] = outa, outb
                nc.tensor.matmul(
                    out=zq[TB:KR, :], lhsT=lhs_it_t[0:KR, 0:49],
                    rhs=sts[ua][0:KR, :], start=True, stop=True)
                nc.tensor.matmul(
                    out=zq[0:49, :], lhsT=lhs_it_t[0:KR, 49:98],
                    rhs=sts[ub][0:KR, :], start=True, stop=True)
                nc.scalar.activation(out=th[0:KR, :], in_=zq[0:KR, :],
                                     func=fn)
                TT(out=outa[TB:KR, :], in0=th[TB:KR, :],
                   in1=ops[ua][TB:KR, :], op=Alu.mult)
                TT(out=outb[0:49, :], in0=th[0:49, :],
                   in1=ops[ub][0:49, :], op=Alu.mult)

        # ---------------- final conv, mask, output per quad
        outbuf = outp.tile([P, 32 * 49], mybir.dt.int32)

        def final(q):
            ua, ub = 2 * q, 2 * q + 1
            with nc.named_scope("final"):
                zf = psF.tile([P, EB], f32, tag="zf", name=f"zf{q}")
                nc.tensor.matmul(
                    out=zf[TB:KR, :], lhsT=lhs_fin_t[TB:KR, :],
                    rhs=mbs[ua][TB:KR, :], start=True, stop=True)
                nc.tensor.matmul(
                    out=zf[0:49, :], lhsT=lhs_fin_t[0:49, :],
                    rhs=mbs[ub][0:49, :], start=True, stop=True)
                mkB = workp.tile([P, EB], bf16, tag="mkB", name=f"mkB{q}")
                TS(out=mkB[0:KR, :], in0=zf[0:KR, :], scalar1=0.0,
                   scalar2=None, op0=Alu.is_gt)
                # one transpose per 128-env block covers BOTH units' mask
                # bands (cols 0:49 = beta, 64:113 = alpha of the output)
                tpM = psM.tile([P, 4 * P], bf16, tag="tpM", name=f"tpM{q}")
                for g in range(4):
                    cb = g * P
                    nc.tensor.transpose(
                        out=tpM[:, cb:cb + KR],
                        in_=mkB[0:KR, cb:cb + P],
                        identity=ident[0:KR, 0:KR])
                for u in (ua, ub):
                    # alpha mask at transposed cols 64:113, beta at 0:49
                    mcol = TB if u % 2 == 0 else 0
                    h = u % 2
                    ob = outbuf[:]
                    mk = tpM[:]
                    sb = slabs[u // 2][:]
                    base = 4 * u
                    out_ap = AP(tensor=ob.tensor,
                                offset=ob.offset + base * 49,
                                ap=[ob.ap[0], [49, 4], [7, 7], [1, 7]])
                    crop_ap = AP(tensor=sb.tensor,
                                 offset=sb.offset + h * 4 * SLOT,
                                 ap=[sb.ap[0], [SLOT, 4], [32, 7], [1, 7]])
                    # mask read straight from the transpose PSUM tile
                    mask_ap = AP(tensor=mk.tensor, offset=mk.offset + mcol,
                                 ap=[mk.ap[0], [P, 4], [7, 7], [1, 7]])
                    TT(out=out_ap, in0=crop_ap, in1=mask_ap, op=Alu.mult)
                    nc.sync.dma_start(
                        out=out[:].rearrange("(p j) f -> p j f", p=P)[
                            :, 4 * u:4 * u + 4, :],
                        in_=outbuf[:].rearrange("p (j f) -> p j f", j=32)[
                            :, 4 * u:4 * u + 4, :])

        for u in range(NU):
            front(u)
        # wavefront order: quad q's layer l at wave q + l, finals at wave
        # q + 6 -- measured best against per-quad-serial and gated variants
        for w in range(2, 10):
            for q in range(4):
                l = w - q
                if 2 <= l <= 5:
                    iter_l(l, q)
                elif l == 6:
                    final(q)

    _split_excess_waits(nc)
    return nc


# ----------------------------------------------------------------- host side
def _conv_matrix(w):
    w = np.asarray(w, np.float32).reshape(3, 3)
    W = np.zeros((49, 49), np.float32)
    for i in range(7):
        for j in range(7):
            for di in (-1, 0, 1):
                for dj in (-1, 0, 1):
                    ii, jj = i + di, j + dj
                    if 0 <= ii < 7 and 0 <= jj < 7:
                        W[i * 7 + j, ii * 7 + jj] = w[di + 1, dj + 1]
    return W


def _pack_variants(g):
    """[n,25,25,3] int32 -> flat [4*n*1024] uint8 (4 rot90s, padded 32x32).

    byte = ch0 | ch1<<2 | ch2<<4 | closed<<6, wall pad byte = 106.
    """
    ch0 = g[..., 0]
    ch2 = g[..., 2]
    closed = ((ch0 == 2) | (ch2 == 1)).astype(np.uint8)
    v = (ch0 | (g[..., 1] << 2) | (ch2 << 4)).astype(np.uint8) | (closed << 6)
    v = np.pad(v, ((0, 0), (5, 5), (5, 5)), constant_values=106)
    vs = [np.ascontiguousarray(np.rot90(v, k, axes=(2, 1))[:, 0:32, 0:32])
          for k in range(4)]
    return np.stack(vs).reshape(-1, 1)


def _host_idx(pos, dirs):
    """Per-env slab start byte offset into the packed variant array."""
    A = np.array([-1, -32, 1, 32], np.int32)
    B = np.array([32, -1, -32, 1], np.int32)
    C = np.array([VARPIX + 87, 2 * VARPIX + 855, 3 * VARPIX + 831, 63],
                 np.int32)
    e = np.arange(NENV, dtype=np.int32)
    idx = e * 1024 + C[dirs] + A[dirs] * pos[:, 0] + B[dirs] * pos[:, 1]
    return np.ascontiguousarray(idx.reshape(P, 32))


def _install_ntff_hook():
    """Register the axon NTFF profile hook that boot() skips when
    antenv.axon_hooks is absent from the image. Trace-path only."""
    import sys
    import types
    if "antenv.axon_hooks" not in sys.modules:
        mod = types.ModuleType("antenv.axon_hooks")
        store = []
        mod.set_axon_ntff_profile_hook = store.append
        mod.get_axon_ntff_profile_hook = lambda: store[-1] if store else None
        import antenv
        sys.modules["antenv.axon_hooks"] = mod
        antenv.axon_hooks = mod
    mod = sys.modules["antenv.axon_hooks"]
    if mod.get_axon_ntff_profile_hook() is None:
        from trn_agent_boot.trn_boot import _ntff_profile_via_ctypes
        hook = _ntff_profile_via_ctypes("/opt/axon/libaxon_pjrt.so")
        if hook is not None:
            mod.set_axon_ntff_profile_hook(hook)
    # zero-egress container: keep artifacts local
    from concourse import bass_utils as _bu
    _bu.upload_artifacts = lambda d: d


_NC_CACHE = []


def kernel(grids, agent_pos, agent_dir, weight):
    grids = np.asarray(grids)
    agent_pos = np.ascontiguousarray(np.asarray(agent_pos, np.int32))
    agent_dir = np.ascontiguousarray(np.asarray(agent_dir, np.int32))
    N = grids.shape[0]
    ncores = 8
    per = N // ncores
    assert per == NENV, (N, NENV)

    W = _conv_matrix(weight)
    lhs_it = np.zeros((KR, 98), np.float32)
    lhs_it[0:49, 0:49] = -0.01 * W          # alpha: closed rows
    lhs_it[TB:KR, 0:49] = W                 # alpha: t rows
    lhs_it[0:49, 49:98] = W                 # beta: t rows
    lhs_it[TB:KR, 49:98] = -0.01 * W        # beta: closed rows
    lhs_fin = np.zeros((KR, 49), np.float32)
    lhs_fin[0:49] = W
    lhs_fin[TB:KR] = W
    lhs_fin = lhs_fin.astype(ml_dtypes.bfloat16)
    w27 = np.zeros((KR, 1), np.float32)
    w27[0:49, 0] = np.tanh(W[:, 27])
    w27[TB:KR, 0] = np.tanh(W[:, 27])

    in_maps = []
    for c in range(ncores):
        sl = slice(c * per, (c + 1) * per)
        in_maps.append({
            "var": _pack_variants(grids[sl]),
            "idxs": _host_idx(agent_pos[sl], agent_dir[sl]),
            "lhs_it": lhs_it,
            "lhs_fin": lhs_fin,
            "w27": w27,
        })

    nc = _NC_CACHE[0] if _NC_CACHE else build_nc()
    if not _NC_CACHE:
        _NC_CACHE.append(nc)

    trace = bool(int(os.environ.get("KERNEL_TRACE", "0")))
    if trace:
        try:
            _install_ntff_hook()
        except Exception as e:  # tracing is best-effort
            print(f"ntff hook install failed: {e}")
    r = run_bass_kernel_spmd(nc, in_maps, core_ids=list(range(ncores)),
                             trace=trace)
    LAST_RESULTS["bass"] = r
    outs = []
    for res in r.results:
        o = res["out"].reshape(per, 49)
        ch = np.stack([o & 3, (o >> 2) & 3, (o >> 4) & 3], axis=-1)
        outs.append(ch.reshape(per, 7, 7, 3).astype(np.int32))
    return np.concatenate(outs, axis=0)

